# revision 1
# baseline (speedup 1.0000x reference)
"""Trainium2 Bass kernel for nn_BatchedGAT (GATv2 + LayerNorm over dense adjacency).

Contract: kernel(**inputs) takes the FULL inputs from reference.setup_inputs()
and returns the FULL [4, 4096, 256] float32 output, running on 8 NeuronCores.

Sharding (hardcoded): core c handles batch b = c // 2, node half h = c % 2
(rows [h*2048, (h+1)*2048) of that batch element). GAT weights replicated.

The runtime used here has a large fixed per-instruction cost, so the kernel
is structured to minimize instruction count:
  1. Setup: xl = x @ Wl + bl (bias folded in via an augmented ones-row in
     the contraction) written to a DRAM table; xr kept in SBUF.
  2. Extraction (per 2 row-tiles): one prefix-scan over the 0/1 adjacency
     gives each edge its within-row rank; one GPSIMD local_scatter
     compacts column indices into dense per-row neighbor lists. Self
     loops are handled analytically on a separate dense path (adjacency
     has a zero diagonal).
  3. One dma_gather per row-tile (4224 xl rows per instruction).
  4. GATv2 attention with exp-softmax (|e| <= ~1, no max subtraction
     needed), masked weighted sum, bias, LayerNorm - all as a few wide
     ops per 2-tile block.
"""

import numpy as np

import concourse.bass as bass
import concourse.bacc as bacc
import concourse.mybir as mybir
from concourse import tile
from concourse.bass_utils import run_bass_kernel_spmd

F32 = mybir.dt.float32
I16 = mybir.dt.int16
Alu = mybir.AluOpType
Act = mybir.ActivationFunctionType
X = mybir.AxisListType.X

B, N, K, IN, H, D = 4, 4096, 32, 64, 4, 64
HD = H * D  # 256
NEG_SLOPE = 0.2
EPS = 1e-5

NCORES = 8
T = N // 2  # 2048 targets per core
NT = T // 128  # 16 target tiles
KS = K + 1  # 33 gather slots per target (<=32 used + pad)
SLOT = KS  # per-row slot stride in the packed neighbor list (33; JJ*SLOT even)
JJ = 2  # row-tiles per extraction/compute block
NBLK = NT // JJ
BIGC = 8192.0  # sentinel for invalid scatter indices
W8 = KS * 8  # wrapped index columns per tile

_cache = {}
TIME_REPEAT = 9


def ap_of(t):
    return t if isinstance(t, bass.AP) else t[:]


def rap(t, pairs, extra_off=0):
    """AP on tile/AP `t`: keep partition dim, set free [step, count] pairs
    (element units), optionally add an element offset."""
    a = ap_of(t)
    return bass.AP(tensor=a.tensor, offset=a.offset + extra_off,
                   ap=[a.ap[0], *pairs])


def build_program(repeat=1):
    nc = bacc.Bacc("TRN2", target_bir_lowering=False, debug=False,
                   num_devices=NCORES)

    adj_d = nc.dram_tensor("adj", [T, N], F32, kind="ExternalInput")
    x_d = nc.dram_tensor("x", [N, IN], F32, kind="ExternalInput")
    xh_d = nc.dram_tensor("xh", [T, IN], F32, kind="ExternalInput")
    wl_d = nc.dram_tensor("Wl", [IN, HD], F32, kind="ExternalInput")
    bl_d = nc.dram_tensor("bl", [HD], F32, kind="ExternalInput")
    wr_d = nc.dram_tensor("Wr", [IN, HD], F32, kind="ExternalInput")
    br_d = nc.dram_tensor("br", [HD], F32, kind="ExternalInput")
    att_d = nc.dram_tensor("attv", [HD], F32, kind="ExternalInput")
    bias_d = nc.dram_tensor("bias", [HD], F32, kind="ExternalInput")
    gamma_d = nc.dram_tensor("gamma", [HD], F32, kind="ExternalInput")
    beta_d = nc.dram_tensor("beta", [HD], F32, kind="ExternalInput")
    base_d = nc.dram_tensor("base", [1, 1], I16, kind="ExternalInput")
    y_d = nc.dram_tensor("y", [T, HD], F32, kind="ExternalOutput")
    xl_d = nc.dram_tensor("xl_scratch", [N, HD], F32)
    wrap_d = nc.dram_tensor("wrap_scratch", [16 * NT * W8], I16)
    sw_d = nc.dram_tensor("selfwrap_scratch", [T], I16)

    with tile.TileContext(nc) as tc:
        _emit(nc, tc, locals(), repeat)
    nc.compile()
    return nc


def _emit(nc, tc, io, repeat):
    adj_d, x_d, xh_d, y_d, xl_d, wrap_d, sw_d = (
        io[k] for k in ("adj_d", "x_d", "xh_d", "y_d", "xl_d", "wrap_d",
                        "sw_d"))

    from contextlib import ExitStack
    ctx = ExitStack()
    with ctx:
        consts = ctx.enter_context(tc.tile_pool(name="consts", bufs=1))

        def bconst(dram_t, tag):
            t = consts.tile([128, HD], F32, tag=tag)
            nc.sync.dma_start(
                out=t[:], in_=bass.AP(tensor=dram_t, offset=0,
                                      ap=[[0, 128], [1, HD]]))
            return t

        att_b = bconst(io["att_d"], "att_b")
        bias_b = bconst(io["bias_d"], "bias_b")
        gamma_b = bconst(io["gamma_d"], "gamma_b")
        beta_b = bconst(io["beta_d"], "beta_b")

        eps_t = consts.tile([128, 1], F32)
        nc.vector.memset(eps_t[:], EPS)
        zero1 = consts.tile([128, 1], F32)
        nc.vector.memset(zero1[:], 0.0)

        iota_tmp = consts.tile([128, KS], I16)
        nc.gpsimd.iota(iota_tmp[:], pattern=[[1, KS]], base=0,
                       channel_multiplier=0)
        iota_kf = consts.tile([128, KS], F32)
        nc.vector.tensor_copy(out=iota_kf[:], in_=iota_tmp[:])

        base_b = consts.tile([128, 1], I16)
        nc.sync.dma_start(out=base_b[:],
                          in_=bass.AP(tensor=io["base_d"], offset=0,
                                      ap=[[0, 128], [1, 1]]))

        # persistent per-core state
        xr_all = consts.tile([128, NT, HD], F32)
        xlh_all = consts.tile([128, NT, HD], F32)
        wrap128 = consts.tile([128, NT * W8], I16)
        nbr_all = consts.tile([128, NT * SLOT], I16)
        cnt_all = consts.tile([128, NT], F32)
        wself_all = consts.tile([128, NT, H], F32)

        # ---- setup: xl table in DRAM (bias via ones-row), xr in SBUF ------
        with tc.tile_pool(name="setup", bufs=2) as setup, \
             tc.tile_pool(name="setup_ps", bufs=4, space="PSUM") as setup_ps:
            xTa = setup.tile([IN + 1, N], F32)
            nc.sync.dma_start(
                out=xTa[:IN, :],
                in_=bass.AP(tensor=x_d, offset=0, ap=[[1, IN], [IN, N]]))
            nc.vector.memset(xTa[IN:IN + 1, :], 1.0)
            xhTa = setup.tile([IN + 1, T], F32)
            nc.sync.dma_start(
                out=xhTa[:IN, :],
                in_=bass.AP(tensor=xh_d, offset=0, ap=[[1, IN], [IN, T]]))
            nc.vector.memset(xhTa[IN:IN + 1, :], 1.0)

            wla = setup.tile([IN + 1, HD], F32)
            nc.sync.dma_start(out=wla[:IN, :], in_=io["wl_d"].ap())
            nc.sync.dma_start(out=wla[IN:IN + 1, :],
                              in_=bass.AP(tensor=io["bl_d"], offset=0,
                                          ap=[[0, 1], [1, HD]]))
            wra = setup.tile([IN + 1, HD], F32)
            nc.sync.dma_start(out=wra[:IN, :], in_=io["wr_d"].ap())
            nc.sync.dma_start(out=wra[IN:IN + 1, :],
                              in_=bass.AP(tensor=io["br_d"], offset=0,
                                          ap=[[0, 1], [1, HD]]))

            for grp in range(N // 512):  # xl -> DRAM, 4 node-chunks per DMA
                ps = setup_ps.tile([128, 4, HD], F32, tag="mm")
                for c4 in range(4):
                    c = grp * 4 + c4
                    nc.tensor.matmul(out=ps[:, c4, :],
                                     lhsT=xTa[:, c * 128:(c + 1) * 128],
                                     rhs=wla[:], start=True, stop=True)
                xls = setup.tile([128, 4, HD], F32, tag="xls")
                nc.vector.tensor_copy(out=xls[:], in_=ps[:])
                nc.sync.dma_start(
                    out=bass.AP(tensor=xl_d, offset=grp * 512 * HD,
                                ap=[[HD, 128], [128 * HD, 4], [1, HD]]),
                    in_=xls[:])

            for grp in range(NT // 4):  # xr, SBUF resident
                ps = setup_ps.tile([128, 4, HD], F32, tag="mm")
                for c4 in range(4):
                    c = grp * 4 + c4
                    nc.tensor.matmul(out=ps[:, c4, :],
                                     lhsT=xhTa[:, c * 128:(c + 1) * 128],
                                     rhs=wra[:], start=True, stop=True)
                nc.vector.tensor_copy(out=xr_all[:, grp * 4:(grp + 1) * 4, :],
                                      in_=ps[:])

        # ---- main (repeatable for timing) ---------------------------------
        smallp = ctx.enter_context(tc.tile_pool(name="smallp", bufs=2))
        outp = ctx.enter_context(tc.tile_pool(name="outp", bufs=2))

        for _rep in range(repeat):
            # ---- phase 1: neighbor extraction per 2-tile block -------------
            p1 = tc.alloc_tile_pool(name="p1", bufs=1)
            adjp = tc.alloc_tile_pool(name="adjp", bufs=2)
            iota_j = p1.tile([128, JJ * N], I16)
            nc.gpsimd.iota(iota_j[:], pattern=[[0, JJ], [1, N]], base=0,
                           channel_multiplier=0)
            for blk in range(NBLK):
                t0 = blk * JJ
                adj_t = adjp.tile([128, JJ, N], F32, tag="adj")
                nc.sync.dma_start(
                    out=adj_t[:],
                    in_=bass.AP(tensor=adj_d, offset=t0 * 128 * N,
                                ap=[[N, 128], [128 * N, JJ], [1, N]]))
                adj_f = adj_t[:].rearrange("p j n -> p (j n)")

                s = p1.tile([128, JJ * N], F32, tag="scan")
                nc.vector.tensor_tensor_scan(
                    out=s[:], data0=adj_f, data1=rap(zero1, [[0, JJ * N]]),
                    initial=0.0, op0=Alu.add, op1=Alu.add)

                rowlast = smallp.tile([128, JJ], F32, tag="rowlast")
                nc.vector.tensor_copy(out=rowlast[:],
                                      in_=rap(s, [[N, JJ]], extra_off=N - 1))
                nc.vector.tensor_copy(out=cnt_all[:, t0:t0 + 1],
                                      in_=rowlast[:, 0:1])
                nc.vector.tensor_tensor(out=cnt_all[:, t0 + 1:t0 + 2],
                                        in0=rowlast[:, 1:2],
                                        in1=rowlast[:, 0:1],
                                        op=Alu.subtract)
                # corr[j] = j*SLOT - 1 - inclusive_total(rows < j)
                corr = smallp.tile([128, JJ], F32, tag="corr")
                nc.vector.memset(corr[:, 0:1], -1.0)
                nc.vector.tensor_scalar(out=corr[:, 1:2], in0=rowlast[:, 0:1],
                                        scalar1=-1.0, scalar2=float(SLOT - 1),
                                        op0=Alu.mult, op1=Alu.add)
                nc.vector.tensor_tensor(out=s[:], in0=s[:],
                                        in1=rap(corr, [[1, JJ], [0, N]]),
                                        op=Alu.add)
                nc.vector.scalar_tensor_tensor(out=s[:], in0=s[:], scalar=BIGC,
                                               op0=Alu.add, in1=adj_f,
                                               op1=Alu.mult)
                sidx = p1.tile([128, JJ * N], I16, tag="sidx")
                nc.vector.tensor_scalar_add(out=sidx[:], in0=s[:],
                                            scalar1=-BIGC)
                nc.gpsimd.local_scatter(
                    out_ap=nbr_all[:, t0 * SLOT:(t0 + JJ) * SLOT],
                    data_ap=iota_j[:], idxs_ap=sidx[:], channels=128,
                    num_elems=JJ * SLOT, num_idxs=JJ * N)

            adjp.release()
            p1.release()

            # ---- wrapped index layout for dma_gather -----------------------
            # edge e = k*128+p of tile t -> wrapped[p%16, t*W8 + k*8 + p//16]
            wrapT = smallp.tile([16, NT, KS, 8], I16, tag="wrapT")
            for ph in range(8):
                nc.sync.dma_start(
                    out=rap(wrapT, [[8, NT * KS]], extra_off=ph),
                    in_=nbr_all[ph * 16:(ph + 1) * 16, :])
            nc.sync.dma_start(out=wrap_d.ap(),
                              in_=wrapT[:].rearrange("p t k e -> p (t k e)"))
            nc.sync.dma_start(
                out=wrap128[:],
                in_=bass.AP(tensor=wrap_d, offset=0,
                            ap=[[0, 8], [NT * W8, 16], [1, NT * W8]]))

            gp = tc.alloc_tile_pool(name="gp", bufs=1)
            auxp = tc.alloc_tile_pool(name="auxp", bufs=1)

            # ---- self path: gather xl rows of own targets ------------------
            # wrapped value at [r, col] = base + r + 16*col
            sw16 = smallp.tile([16, T // 16], I16, tag="sw16")
            nc.gpsimd.iota(sw16[:], pattern=[[16, T // 16]], base=0,
                           channel_multiplier=1)
            nc.vector.tensor_tensor(out=sw16[:], in0=sw16[:],
                                    in1=rap(base_b[:16, :], [[0, T // 16]]),
                                    op=Alu.add)
            nc.sync.dma_start(out=sw_d.ap(), in_=sw16[:])
            swf = smallp.tile([128, T // 16], I16, tag="swf")
            nc.sync.dma_start(
                out=swf[:],
                in_=bass.AP(tensor=sw_d, offset=0,
                            ap=[[0, 8], [T // 16, 16], [1, T // 16]]))
            nc.gpsimd.dma_gather(
                out_ap=xlh_all[:], in_ap=xl_d.ap(), idxs_ap=swf[:],
                num_idxs=T, num_idxs_reg=T, elem_size=HD, single_packet=False)

            # self scores: w_self = exp(<att, lrelu(xlh + xr)>)
            zs = auxp.tile([128, NT, HD], F32, tag="aux")
            nc.vector.tensor_tensor(out=zs[:], in0=xlh_all[:], in1=xr_all[:],
                                    op=Alu.add)
            nc.vector.scalar_tensor_tensor(out=zs[:], in0=zs[:],
                                           scalar=NEG_SLOPE, op0=Alu.mult,
                                           in1=zs[:], op1=Alu.max)
            nc.vector.tensor_tensor(out=zs[:], in0=zs[:],
                                    in1=rap(att_b, [[0, NT], [1, HD]]),
                                    op=Alu.mult)
            es = smallp.tile([128, NT * H], F32, tag="eself")
            nc.vector.tensor_reduce(
                out=es[:], in_=rap(zs, [[D, NT * H], [1, D]]),
                op=Alu.add, axis=X)
            nc.scalar.activation(out=wself_all[:], in_=es[:], func=Act.Exp)

            # ---- phase 2: gather + attention per 2-tile block --------------
            for blk in range(NBLK):
                t0 = blk * JJ
                g = gp.tile([128, JJ, KS, HD], F32, tag="g")
                for jj in range(JJ):
                    t = t0 + jj
                    nc.gpsimd.dma_gather(
                        out_ap=g[:, jj, :, :], in_ap=xl_d.ap(),
                        idxs_ap=wrap128[:, t * W8:(t + 1) * W8],
                        num_idxs=KS * 128, num_idxs_reg=KS * 128,
                        elem_size=HD, single_packet=False)

                e = smallp.tile([128, JJ, KS * H], F32, tag="e")
                for jj in range(JJ):
                    aux = auxp.tile([128, KS, HD], F32, tag="aux")
                    nc.vector.tensor_tensor(
                        out=aux[:], in0=g[:, jj, :, :],
                        in1=rap(xr_all, [[0, KS], [1, HD]],
                                extra_off=(t0 + jj) * HD),
                        op=Alu.add)
                    nc.vector.scalar_tensor_tensor(
                        out=aux[:], in0=aux[:], scalar=NEG_SLOPE,
                        op0=Alu.mult, in1=aux[:], op1=Alu.max)
                    nc.vector.tensor_tensor(
                        out=aux[:], in0=aux[:],
                        in1=rap(att_b, [[0, KS], [1, HD]]), op=Alu.mult)
                    nc.vector.tensor_reduce(
                        out=e[:, jj, :], in_=rap(aux, [[D, KS * H], [1, D]]),
                        op=Alu.add, axis=X)
                w = smallp.tile([128, JJ, KS, H], F32, tag="w")
                nc.scalar.activation(out=w[:], in_=e[:], func=Act.Exp)
                kmask = smallp.tile([128, JJ, KS], F32, tag="kmask")
                nc.vector.tensor_tensor(
                    out=kmask[:], in0=rap(iota_kf, [[0, JJ], [1, KS]]),
                    in1=rap(cnt_all, [[1, JJ], [0, KS]], extra_off=t0),
                    op=Alu.is_lt)
                nc.vector.tensor_tensor(
                    out=w[:], in0=w[:],
                    in1=rap(kmask, [[KS, JJ], [1, KS], [0, H]]),
                    op=Alu.mult)
                zsum = smallp.tile([128, JJ, H], F32, tag="zsum")
                nc.vector.tensor_reduce(
                    out=zsum[:], in_=rap(w, [[KS * H, JJ], [1, H], [H, KS]]),
                    op=Alu.add, axis=X)
                nc.vector.tensor_tensor(out=zsum[:], in0=zsum[:],
                                        in1=wself_all[:, t0:t0 + JJ, :],
                                        op=Alu.add)
                rz = smallp.tile([128, JJ, H], F32, tag="rz")
                nc.vector.reciprocal(out=rz[:], in_=zsum[:])
                nc.vector.tensor_tensor(
                    out=w[:], in0=w[:],
                    in1=rap(rz, [[H, JJ], [0, KS], [1, H]]),
                    op=Alu.mult)
                o = outp.tile([128, JJ, HD], F32, tag="o")
                for jj in range(JJ):
                    nc.vector.tensor_tensor(
                        out=g[:, jj, :, :], in0=g[:, jj, :, :],
                        in1=rap(w[:, jj, :, :], [[H, KS], [1, H], [0, D]]),
                        op=Alu.mult)
                    nc.vector.tensor_reduce(
                        out=o[:, jj, :],
                        in_=rap(g[:, jj, :, :],
                                [[D, H], [1, D], [H * D, KS]]),
                        op=Alu.add, axis=X)
                # self contribution: o += xlh * (wself * rz)
                wsn = smallp.tile([128, JJ, H], F32, tag="wsn")
                nc.vector.tensor_tensor(out=wsn[:],
                                        in0=wself_all[:, t0:t0 + JJ, :],
                                        in1=rz[:], op=Alu.mult)
                sc = smallp.tile([128, JJ, HD], F32, tag="sc")
                nc.vector.tensor_tensor(
                    out=sc[:], in0=xlh_all[:, t0:t0 + JJ, :],
                    in1=rap(wsn, [[H, JJ], [1, H], [0, D]]), op=Alu.mult)
                nc.vector.tensor_tensor(out=o[:], in0=o[:], in1=sc[:],
                                        op=Alu.add)
                nc.vector.tensor_tensor(
                    out=o[:], in0=o[:],
                    in1=rap(bias_b, [[0, JJ], [1, HD]]), op=Alu.add)

                # LayerNorm over HD
                stats = smallp.tile([128, JJ, 6], F32, tag="stats")
                mv = smallp.tile([128, JJ, 2], F32, tag="mv")
                for jj in range(JJ):
                    nc.vector.bn_stats(out=stats[:, jj, :], in_=o[:, jj, :])
                    nc.vector.bn_aggr(out=mv[:, jj, :], in_=stats[:, jj, :])
                ve = smallp.tile([128, JJ], F32, tag="ve")
                nc.vector.tensor_tensor(out=ve[:],
                                        in0=rap(mv, [[2, JJ]], extra_off=1),
                                        in1=rap(eps_t, [[0, JJ]]), op=Alu.add)
                lnv = smallp.tile([128, JJ], F32, tag="lnv")
                nc.scalar.activation(out=lnv[:], in_=ve[:], func=Act.Ln)
                rstd = smallp.tile([128, JJ], F32, tag="rstd")
                nc.scalar.activation(out=rstd[:], in_=lnv[:], func=Act.Exp,
                                     scale=-0.5)
                for jj in range(JJ):
                    nc.vector.scalar_tensor_tensor(
                        out=o[:, jj, :], in0=o[:, jj, :],
                        scalar=mv[:, jj, 0:1], op0=Alu.subtract,
                        in1=rap(rstd, [[0, HD]], extra_off=jj), op1=Alu.mult)
                nc.vector.tensor_tensor(
                    out=o[:], in0=o[:],
                    in1=rap(gamma_b, [[0, JJ], [1, HD]]), op=Alu.mult)
                nc.vector.tensor_tensor(
                    out=o[:], in0=o[:],
                    in1=rap(beta_b, [[0, JJ], [1, HD]]), op=Alu.add)
                nc.sync.dma_start(
                    out=bass.AP(tensor=y_d, offset=t0 * 128 * HD,
                                ap=[[HD, 128], [128 * HD, JJ], [1, HD]]),
                    in_=o[:])
            auxp.release()
            gp.release()


def make_in_maps(inputs):
    adj = np.ascontiguousarray(inputs["adj"], np.float32)
    x = np.ascontiguousarray(inputs["x"], np.float32)
    flat = {
        "Wl": np.ascontiguousarray(inputs["Wl"], np.float32),
        "bl": np.ascontiguousarray(inputs["bl"], np.float32),
        "Wr": np.ascontiguousarray(inputs["Wr"], np.float32),
        "br": np.ascontiguousarray(inputs["br"], np.float32),
        "attv": np.ascontiguousarray(inputs["att"], np.float32).reshape(HD),
        "bias": np.ascontiguousarray(inputs["bias"], np.float32),
        "gamma": np.ascontiguousarray(inputs["gamma"], np.float32),
        "beta": np.ascontiguousarray(inputs["beta"], np.float32),
    }
    in_maps = []
    for c in range(NCORES):
        b, h = c // 2, c % 2
        in_maps.append({
            "adj": np.ascontiguousarray(adj[b, h * T:(h + 1) * T, :]),
            "x": np.ascontiguousarray(x[b]),
            "xh": np.ascontiguousarray(x[b, h * T:(h + 1) * T, :]),
            "base": np.array([[h * T]], np.int16),
            **flat,
        })
    return in_maps


def kernel(**inputs) -> np.ndarray:
    if "nc" not in _cache:
        _cache["nc"] = build_program()
    nc = _cache["nc"]
    res = run_bass_kernel_spmd(nc, make_in_maps(inputs), list(range(NCORES)))
    y = np.zeros((B, N, HD), np.float32)
    for c in range(NCORES):
        b, h = c // 2, c % 2
        y[b, h * T:(h + 1) * T, :] = res.results[c]["y"]
    return y



# revision 29
# speedup vs baseline: 26.1673x; 26.1673x over previous
"""Trainium2 Bass kernel for nn_BatchedGAT (GATv2 + LayerNorm over dense adjacency).

Contract: kernel(**inputs) takes the FULL inputs from reference.setup_inputs()
and returns the FULL [4, 4096, 256] float32 output, running on 8 NeuronCores.

Sharding (hardcoded): core c handles batch b = c // 2, node half h = c % 2
(rows [h*2048, (h+1)*2048) of that batch element). GAT weights replicated.

v2 design (per core, per repetition):
  1. Setup: xl = x @ Wl + bl written to a bf16 DRAM gather table; xr and the
     core's own xl rows (self features) computed by PE straight into SBUF
     (bf16) - no self-gather needed.
  2. Extraction per 2-row-tile block: adjacency loaded with a casting SWDGE
     DMA to bf16, one prefix-scan PER ROW with initial=j*SLOT (bakes the
     slot offset into the count - no correction pass), then
     sidx = scan*adj - 1 (invalid slots -> -1, ignored by local_scatter)
     compacted into dense per-row neighbor lists by GPSIMD local_scatter.
  3. One bf16 dma_gather per row-tile (4224 rows x 512B), alternating SWDGE
     queues so two gathers run concurrently.
  4. GATv2 attention with exp-softmax (|e| small, no max subtraction),
     bf16 elementwise math, f32 softmax/LayerNorm, wide fused-AP ops.
"""

import numpy as np

import concourse.bass as bass
import concourse.bacc as bacc
import concourse.mybir as mybir
from concourse import tile
from concourse.bass_utils import run_bass_kernel_spmd

F32 = mybir.dt.float32
BF16 = mybir.dt.bfloat16
I16 = mybir.dt.int16
Alu = mybir.AluOpType
Act = mybir.ActivationFunctionType
X = mybir.AxisListType.X

B, N, K, IN, H, D = 4, 4096, 32, 64, 4, 64
HD = H * D  # 256
NEG_SLOPE = 0.2
EPS = 1e-5

NCORES = 8
T = N // 2  # 2048 targets per core
NT = T // 128  # 16 target tiles
KS = K + 1  # 33 gather slots per target (<=32 used + pad)
SLOT = KS  # per-row slot stride in the packed neighbor list
JJ = 2  # row-tiles per block
NBLK = NT // JJ
W8 = KS * 8  # wrapped index columns per tile

_cache = {}
TIME_REPEAT = 9
BEST_MODE = "full"  # production variant used by kernel(); timing uses it too


def ap_of(t):
    return t if isinstance(t, bass.AP) else t[:]


def rap(t, pairs, extra_off=0):
    """AP on tile/AP `t`: keep partition dim, set free [step, count] pairs
    (element units), optionally add an element offset."""
    a = ap_of(t)
    return bass.AP(tensor=a.tensor, offset=a.offset + extra_off,
                   ap=[a.ap[0], *pairs])


def build_program(repeat=1, mode="full"):
    nc = bacc.Bacc("TRN2", target_bir_lowering=False, debug=False,
                   num_devices=NCORES,
                   num_swdge_queues=2 if mode == "dq" else 1)

    adj_d = nc.dram_tensor("adj", [T, N], F32, kind="ExternalInput")
    x_d = nc.dram_tensor("x", [N, IN], F32, kind="ExternalInput")
    xh_d = nc.dram_tensor("xh", [T, IN], F32, kind="ExternalInput")
    wl_d = nc.dram_tensor("Wl", [IN, HD], F32, kind="ExternalInput")
    bl_d = nc.dram_tensor("bl", [HD], F32, kind="ExternalInput")
    wr_d = nc.dram_tensor("Wr", [IN, HD], F32, kind="ExternalInput")
    br_d = nc.dram_tensor("br", [HD], F32, kind="ExternalInput")
    att_d = nc.dram_tensor("attv", [HD], F32, kind="ExternalInput")
    bias_d = nc.dram_tensor("bias", [HD], F32, kind="ExternalInput")
    gamma_d = nc.dram_tensor("gamma", [HD], F32, kind="ExternalInput")
    beta_d = nc.dram_tensor("beta", [HD], F32, kind="ExternalInput")
    y_d = nc.dram_tensor("y", [T, HD], F32, kind="ExternalOutput")
    xl_d = nc.dram_tensor("xl_scratch", [N, HD], BF16)
    wrap_d = nc.dram_tensor("wrap_scratch", [16 * NT * W8], I16)

    with tile.TileContext(nc) as tc:
        _emit(nc, tc, locals(), repeat, mode)
    nc.compile()
    return nc


def _emit(nc, tc, io, repeat, mode="full"):
    adj_d, x_d, xh_d, y_d, xl_d, wrap_d = (
        io[k] for k in ("adj_d", "x_d", "xh_d", "y_d", "xl_d", "wrap_d"))

    from contextlib import ExitStack
    ctx = ExitStack()
    with ctx:
        consts = ctx.enter_context(tc.tile_pool(name="consts", bufs=1))

        def bconst(dram_t, tag, dt=F32):
            t = consts.tile([128, HD], dt, tag=tag)
            if dt == F32:
                nc.sync.dma_start(
                    out=t[:], in_=bass.AP(tensor=dram_t, offset=0,
                                          ap=[[0, 128], [1, HD]]))
            else:
                stage = consts.tile([128, HD], F32, tag=tag + "_st")
                nc.sync.dma_start(
                    out=stage[:], in_=bass.AP(tensor=dram_t, offset=0,
                                              ap=[[0, 128], [1, HD]]))
                nc.vector.tensor_copy(out=t[:], in_=stage[:])
            return t

        att_bb = bconst(io["att_d"], "att_bb", BF16)
        bias_b = bconst(io["bias_d"], "bias_b")
        gamma_b = bconst(io["gamma_d"], "gamma_b")
        beta_b = bconst(io["beta_d"], "beta_b")

        eps_t = consts.tile([128, 1], F32)
        nc.vector.memset(eps_t[:], EPS)
        zerob = consts.tile([128, 1], BF16)
        nc.vector.memset(zerob[:], 0.0)

        iota_tmp = consts.tile([128, KS], I16)
        nc.gpsimd.iota(iota_tmp[:], pattern=[[1, KS]], base=0,
                       channel_multiplier=0)
        iota_kf = consts.tile([128, KS], F32)
        nc.vector.tensor_copy(out=iota_kf[:], in_=iota_tmp[:])

        iota_j = consts.tile([128, JJ * N], I16)
        nc.gpsimd.iota(iota_j[:], pattern=[[0, JJ], [1, N]], base=0,
                       channel_multiplier=0)

        gd = None
        if mode == "nodep":
            gd = consts.tile([128, KS, HD], BF16)
            nc.vector.memset(gd[:], 0.25)

        # persistent per-core state
        xr_all = consts.tile([128, NT, HD], BF16)
        xlh_all = consts.tile([128, NT, HD], BF16)
        wrap128 = consts.tile([128, NT * W8], I16)
        nbr_all = consts.tile([128, NT * SLOT], I16)
        cnt_all = consts.tile([128, NT], F32)
        wself_all = consts.tile([128, NT, H], F32)

        # ---- setup: xl table in DRAM (bias via ones-row); xr, xlh on PE ----
        with tc.tile_pool(name="setup", bufs=2) as setup, \
             tc.tile_pool(name="setup_ps", bufs=4, space="PSUM") as setup_ps:
            xTa = setup.tile([IN + 1, N], F32)
            nc.sync.dma_start(
                out=xTa[:IN, :],
                in_=bass.AP(tensor=x_d, offset=0, ap=[[1, IN], [IN, N]]))
            nc.vector.memset(xTa[IN:IN + 1, :], 1.0)
            xhTa = setup.tile([IN + 1, T], F32)
            nc.sync.dma_start(
                out=xhTa[:IN, :],
                in_=bass.AP(tensor=xh_d, offset=0, ap=[[1, IN], [IN, T]]))
            nc.vector.memset(xhTa[IN:IN + 1, :], 1.0)

            wla = setup.tile([IN + 1, HD], F32)
            nc.sync.dma_start(out=wla[:IN, :], in_=io["wl_d"].ap())
            nc.sync.dma_start(out=wla[IN:IN + 1, :],
                              in_=bass.AP(tensor=io["bl_d"], offset=0,
                                          ap=[[0, 1], [1, HD]]))
            wra = setup.tile([IN + 1, HD], F32)
            nc.sync.dma_start(out=wra[:IN, :], in_=io["wr_d"].ap())
            nc.sync.dma_start(out=wra[IN:IN + 1, :],
                              in_=bass.AP(tensor=io["br_d"], offset=0,
                                          ap=[[0, 1], [1, HD]]))

            for grp in range(N // 512):  # xl -> DRAM, 4 node-chunks per DMA
                ps = setup_ps.tile([128, 4, HD], F32, tag="mm")
                for c4 in range(4):
                    c = grp * 4 + c4
                    nc.tensor.matmul(out=ps[:, c4, :],
                                     lhsT=xTa[:, c * 128:(c + 1) * 128],
                                     rhs=wla[:], start=True, stop=True)
                xls = setup.tile([128, 4, HD], BF16, tag="xls")
                nc.vector.tensor_copy(out=xls[:], in_=ps[:])
                nc.sync.dma_start(
                    out=bass.AP(tensor=xl_d, offset=grp * 512 * HD,
                                ap=[[HD, 128], [128 * HD, 4], [1, HD]]),
                    in_=xls[:])

            for grp in range(NT // 4):  # xr and xlh (self rows), SBUF bf16
                ps = setup_ps.tile([128, 4, HD], F32, tag="mm")
                for c4 in range(4):
                    c = grp * 4 + c4
                    nc.tensor.matmul(out=ps[:, c4, :],
                                     lhsT=xhTa[:, c * 128:(c + 1) * 128],
                                     rhs=wra[:], start=True, stop=True)
                nc.vector.tensor_copy(out=xr_all[:, grp * 4:(grp + 1) * 4, :],
                                      in_=ps[:])
                ps2 = setup_ps.tile([128, 4, HD], F32, tag="mm")
                for c4 in range(4):
                    c = grp * 4 + c4
                    nc.tensor.matmul(out=ps2[:, c4, :],
                                     lhsT=xhTa[:, c * 128:(c + 1) * 128],
                                     rhs=wla[:], start=True, stop=True)
                nc.vector.tensor_copy(out=xlh_all[:, grp * 4:(grp + 1) * 4, :],
                                      in_=ps2[:])

        # ---- main (repeatable for timing) ---------------------------------
        smallp = ctx.enter_context(tc.tile_pool(name="smallp", bufs=2))
        outp = ctx.enter_context(tc.tile_pool(name="outp", bufs=2))

        for _rep in range(repeat):
            # ---- phase 1: neighbor extraction per 2-tile block -------------
            p1 = tc.alloc_tile_pool(name="p1", bufs=2)
            adjp = tc.alloc_tile_pool(name="adjp", bufs=3)
            for blk in range(NBLK):
                t0 = blk * JJ
                adj_t = [adjp.tile([128, N], BF16, tag="adj",
                                   name=f"adj{blk}_{j}") for j in (0, 1)]
                if mode in ("synadj", "dq"):
                    for j in range(JJ):
                        adj_f32 = adjp.tile([128, N], F32, tag="adjf",
                                            name=f"adjf{blk}_{j}")
                        nc.sync.dma_start(
                            out=adj_f32[:],
                            in_=bass.AP(tensor=adj_d,
                                        offset=(t0 + j) * 128 * N,
                                        ap=[[N, 128], [1, N]]))
                        nc.vector.tensor_copy(out=adj_t[j][:],
                                              in_=adj_f32[:])
                else:
                    for j in range(JJ):
                        nc.gpsimd.dma_start(
                            out=adj_t[j][:],
                            in_=bass.AP(tensor=adj_d,
                                        offset=(t0 + j) * 128 * N,
                                        ap=[[N, 128], [1, N]]))

                s = p1.tile([128, JJ, N], BF16, tag="scan")
                for j in range(JJ):
                    nc.vector.tensor_tensor_scan(
                        out=s[:, j, :], data0=adj_t[j][:],
                        data1=rap(zerob, [[0, N]]),
                        initial=float(j * SLOT), op0=Alu.add, op1=Alu.add)
                s_f = s[:].rearrange("p j n -> p (j n)")

                rowlast = smallp.tile([128, JJ], F32, tag="rowlast")
                nc.vector.tensor_copy(out=rowlast[:],
                                      in_=rap(s_f, [[N, JJ]],
                                              extra_off=N - 1))
                nc.vector.tensor_copy(out=cnt_all[:, t0:t0 + 1],
                                      in_=rowlast[:, 0:1])
                nc.vector.tensor_scalar_add(out=cnt_all[:, t0 + 1:t0 + 2],
                                            in0=rowlast[:, 1:2],
                                            scalar1=-float(SLOT))
                # sidx = s*adj - 1: valid -> rank-1 + j*SLOT, invalid -> -1
                for j in range(JJ):
                    nc.vector.tensor_tensor(out=s[:, j, :], in0=s[:, j, :],
                                            in1=adj_t[j][:], op=Alu.mult)
                sidx = s[:].bitcast(I16).rearrange("p j n -> p (j n)")
                nc.vector.tensor_scalar_add(out=sidx, in0=s_f, scalar1=-1.0)
                nc.gpsimd.local_scatter(
                    out_ap=nbr_all[:, t0 * SLOT:(t0 + JJ) * SLOT],
                    data_ap=iota_j[:], idxs_ap=sidx, channels=128,
                    num_elems=JJ * SLOT, num_idxs=JJ * N)

            adjp.release()
            p1.release()

            if mode == "p1only":
                oc = outp.tile([128, NT], F32, tag="oc")
                nc.vector.tensor_copy(out=oc[:], in_=cnt_all[:])
                nc.sync.dma_start(
                    out=bass.AP(tensor=y_d, offset=0,
                                ap=[[HD, 128], [1, NT]]),
                    in_=oc[:])
                continue

            # ---- wrapped index layout for dma_gather -----------------------
            # edge e = k*128+p of tile t -> wrapped[p%16, t*W8 + k*8 + p//16]
            wrapT = smallp.tile([16, NT, KS, 8], I16, tag="wrapT")
            for ph in range(8):
                nc.sync.dma_start(
                    out=rap(wrapT, [[8, NT * KS]], extra_off=ph),
                    in_=nbr_all[ph * 16:(ph + 1) * 16, :])
            nc.sync.dma_start(out=wrap_d.ap(),
                              in_=wrapT[:].rearrange("p t k e -> p (t k e)"))
            nc.sync.dma_start(
                out=wrap128[:],
                in_=bass.AP(tensor=wrap_d, offset=0,
                            ap=[[0, 8], [NT * W8, 16], [1, NT * W8]]))

            gpA = tc.alloc_tile_pool(name="gpA", bufs=2)
            gpB = tc.alloc_tile_pool(name="gpB", bufs=2)
            auxp = tc.alloc_tile_pool(name="auxp", bufs=1)

            # ---- self scores: w_self = exp(<att, lrelu(xlh + xr)>) ---------
            if mode == "novec":
                nc.vector.memset(wself_all[:], 1.0)
            else:
                zs = smallp.tile([128, NT, HD], BF16, tag="zs")
                nc.vector.tensor_tensor(out=zs[:], in0=xlh_all[:],
                                        in1=xr_all[:], op=Alu.add)
                nc.vector.scalar_tensor_tensor(out=zs[:], in0=zs[:],
                                               scalar=NEG_SLOPE, op0=Alu.mult,
                                               in1=zs[:], op1=Alu.max)
                nc.vector.tensor_tensor(out=zs[:], in0=zs[:],
                                        in1=rap(att_bb, [[0, NT], [1, HD]]),
                                        op=Alu.mult)
                es = smallp.tile([128, NT * H], F32, tag="eself")
                nc.vector.tensor_reduce(
                    out=es[:], in_=rap(zs, [[D, NT * H], [1, D]]),
                    op=Alu.add, axis=X)
                nc.scalar.activation(out=wself_all[:], in_=es[:],
                                     func=Act.Exp)

            # ---- phase 2: gather + attention per 2-tile block --------------
            for blk in range(NBLK):
                t0 = blk * JJ
                gpp = gpA if blk % 2 == 0 else gpB
                gt = [gpp.tile([128, KS, HD], BF16, tag="g",
                               name=f"g{blk}_{jj}") for jj in range(JJ)]
                gj = [gt[jj][:] for jj in range(JJ)]
                if mode == "nogather":
                    if _rep == 0 and blk < 2:
                        for jj in range(JJ):
                            nc.vector.memset(gt[jj][:], 0.25)
                else:
                    for jj in range(JJ):
                        t = t0 + jj
                        nc.gpsimd.dma_gather(
                            out_ap=gj[jj], in_ap=xl_d.ap(),
                            idxs_ap=wrap128[:, t * W8:(t + 1) * W8],
                            num_idxs=KS * 128, num_idxs_reg=KS * 128,
                            elem_size=HD, single_packet=False,
                            queue_num=jj if mode == "dq" else 0)

                if mode == "novec":
                    o = outp.tile([128, JJ, HD], F32, tag="o")
                    for jj in range(JJ):
                        nc.vector.tensor_copy(out=o[:, jj, :],
                                              in_=gt[jj][:, 0, :])
                    nc.sync.dma_start(
                        out=bass.AP(tensor=y_d, offset=t0 * 128 * HD,
                                    ap=[[HD, 128], [128 * HD, JJ], [1, HD]]),
                        in_=o[:])
                    continue

                if mode == "nodep":
                    gj = [gd[:], gd[:]]
                e = smallp.tile([128, JJ, KS * H], F32, tag="e")
                for jj in range(JJ):
                    aux = auxp.tile([128, KS, HD], BF16, tag="aux")
                    nc.vector.tensor_tensor(
                        out=aux[:], in0=gj[jj],
                        in1=rap(xr_all, [[0, KS], [1, HD]],
                                extra_off=(t0 + jj) * HD),
                        op=Alu.add)
                    nc.vector.scalar_tensor_tensor(
                        out=aux[:], in0=aux[:], scalar=NEG_SLOPE,
                        op0=Alu.mult, in1=aux[:], op1=Alu.max)
                    nc.vector.tensor_tensor(
                        out=aux[:], in0=aux[:],
                        in1=rap(att_bb, [[0, KS], [1, HD]]),
                        op=Alu.mult)
                    nc.vector.tensor_reduce(
                        out=e[:, jj, :],
                        in_=rap(aux, [[D, KS * H], [1, D]]),
                        op=Alu.add, axis=X)
                w = smallp.tile([128, JJ, KS, H], F32, tag="w")
                nc.scalar.activation(out=w[:], in_=e[:], func=Act.Exp)
                kmask = smallp.tile([128, JJ, KS], F32, tag="kmask")
                nc.vector.tensor_tensor(
                    out=kmask[:], in0=rap(iota_kf, [[0, JJ], [1, KS]]),
                    in1=rap(cnt_all, [[1, JJ], [0, KS]], extra_off=t0),
                    op=Alu.is_lt)
                nc.vector.tensor_tensor(
                    out=w[:], in0=w[:],
                    in1=rap(kmask, [[KS, JJ], [1, KS], [0, H]]),
                    op=Alu.mult)
                zsum = smallp.tile([128, JJ, H], F32, tag="zsum")
                nc.vector.tensor_reduce(
                    out=zsum[:], in_=rap(w, [[KS * H, JJ], [1, H], [H, KS]]),
                    op=Alu.add, axis=X)
                nc.vector.tensor_tensor(out=zsum[:], in0=zsum[:],
                                        in1=wself_all[:, t0:t0 + JJ, :],
                                        op=Alu.add)
                rz = smallp.tile([128, JJ, H], F32, tag="rz")
                nc.vector.reciprocal(out=rz[:], in_=zsum[:])
                w2b = smallp.tile([128, JJ, KS, H], BF16, tag="w2b")
                nc.vector.tensor_tensor(
                    out=w2b[:], in0=w[:],
                    in1=rap(rz, [[H, JJ], [0, KS], [1, H]]),
                    op=Alu.mult)
                o = outp.tile([128, JJ, HD], F32, tag="o")
                for jj in range(JJ):
                    nc.vector.tensor_tensor(
                        out=gj[jj], in0=gj[jj],
                        in1=rap(w2b[:, jj, :, :], [[H, KS], [1, H], [0, D]]),
                        op=Alu.mult)
                    nc.vector.tensor_reduce(
                        out=o[:, jj, :],
                        in_=rap(gj[jj],
                                [[D, H], [1, D], [H * D, KS]]),
                        op=Alu.add, axis=X)
                # self contribution: o += xlh * (wself * rz)
                wsn = smallp.tile([128, JJ, H], BF16, tag="wsn")
                nc.vector.tensor_tensor(out=wsn[:],
                                        in0=wself_all[:, t0:t0 + JJ, :],
                                        in1=rz[:], op=Alu.mult)
                sc = smallp.tile([128, JJ, HD], F32, tag="sc")
                nc.vector.tensor_tensor(
                    out=sc[:], in0=xlh_all[:, t0:t0 + JJ, :],
                    in1=rap(wsn, [[H, JJ], [1, H], [0, D]]), op=Alu.mult)
                nc.vector.tensor_tensor(out=o[:], in0=o[:], in1=sc[:],
                                        op=Alu.add)
                nc.vector.tensor_tensor(
                    out=o[:], in0=o[:],
                    in1=rap(bias_b, [[0, JJ], [1, HD]]), op=Alu.add)

                # LayerNorm over HD
                stats = smallp.tile([128, JJ, 6], F32, tag="stats")
                mv = smallp.tile([128, JJ, 2], F32, tag="mv")
                for jj in range(JJ):
                    nc.vector.bn_stats(out=stats[:, jj, :], in_=o[:, jj, :])
                    nc.vector.bn_aggr(out=mv[:, jj, :], in_=stats[:, jj, :])
                ve = smallp.tile([128, JJ], F32, tag="ve")
                nc.vector.tensor_tensor(out=ve[:],
                                        in0=rap(mv, [[2, JJ]], extra_off=1),
                                        in1=rap(eps_t, [[0, JJ]]), op=Alu.add)
                lnv = smallp.tile([128, JJ], F32, tag="lnv")
                nc.scalar.activation(out=lnv[:], in_=ve[:], func=Act.Ln)
                rstd = smallp.tile([128, JJ], F32, tag="rstd")
                nc.scalar.activation(out=rstd[:], in_=lnv[:], func=Act.Exp,
                                     scale=-0.5)
                for jj in range(JJ):
                    nc.vector.scalar_tensor_tensor(
                        out=o[:, jj, :], in0=o[:, jj, :],
                        scalar=mv[:, jj, 0:1], op0=Alu.subtract,
                        in1=rap(rstd, [[0, HD]], extra_off=jj), op1=Alu.mult)
                nc.vector.tensor_tensor(
                    out=o[:], in0=o[:],
                    in1=rap(gamma_b, [[0, JJ], [1, HD]]), op=Alu.mult)
                nc.vector.tensor_tensor(
                    out=o[:], in0=o[:],
                    in1=rap(beta_b, [[0, JJ], [1, HD]]), op=Alu.add)
                nc.sync.dma_start(
                    out=bass.AP(tensor=y_d, offset=t0 * 128 * HD,
                                ap=[[HD, 128], [128 * HD, JJ], [1, HD]]),
                    in_=o[:])
            auxp.release()
            gpB.release()
            gpA.release()


def make_in_maps(inputs):
    adj = np.ascontiguousarray(inputs["adj"], np.float32)
    x = np.ascontiguousarray(inputs["x"], np.float32)
    flat = {
        "Wl": np.ascontiguousarray(inputs["Wl"], np.float32),
        "bl": np.ascontiguousarray(inputs["bl"], np.float32),
        "Wr": np.ascontiguousarray(inputs["Wr"], np.float32),
        "br": np.ascontiguousarray(inputs["br"], np.float32),
        "attv": np.ascontiguousarray(inputs["att"], np.float32).reshape(HD),
        "bias": np.ascontiguousarray(inputs["bias"], np.float32),
        "gamma": np.ascontiguousarray(inputs["gamma"], np.float32),
        "beta": np.ascontiguousarray(inputs["beta"], np.float32),
    }
    in_maps = []
    for c in range(NCORES):
        b, h = c // 2, c % 2
        in_maps.append({
            "adj": np.ascontiguousarray(adj[b, h * T:(h + 1) * T, :]),
            "x": np.ascontiguousarray(x[b]),
            "xh": np.ascontiguousarray(x[b, h * T:(h + 1) * T, :]),
            **flat,
        })
    return in_maps


def kernel(**inputs) -> np.ndarray:
    if "nc" not in _cache:
        _cache["nc"] = build_program(mode=BEST_MODE)
    nc = _cache["nc"]
    res = run_bass_kernel_spmd(nc, make_in_maps(inputs), list(range(NCORES)))
    y = np.zeros((B, N, HD), np.float32)
    for c in range(NCORES):
        b, h = c // 2, c % 2
        y[b, h * T:(h + 1) * T, :] = res.results[c]["y"]
    return y


# revision 32
# speedup vs baseline: 30.3515x; 1.1599x over previous
"""Trainium2 Bass kernel for nn_BatchedGAT (GATv2 + LayerNorm over dense adjacency).

Contract: kernel(**inputs) takes the FULL inputs from reference.setup_inputs()
and returns the FULL [4, 4096, 256] float32 output, running on 8 NeuronCores.

Sharding (hardcoded): core c handles batch b = c // 2, node half h = c % 2
(rows [h*2048, (h+1)*2048) of that batch element). GAT weights replicated.

v2 design (per core, per repetition):
  1. Setup: xl = x @ Wl + bl written to a bf16 DRAM gather table; xr and the
     core's own xl rows (self features) computed by PE straight into SBUF
     (bf16) - no self-gather needed.
  2. Extraction per 2-row-tile block: adjacency loaded with a casting SWDGE
     DMA to bf16, one prefix-scan PER ROW with initial=j*SLOT (bakes the
     slot offset into the count - no correction pass), then
     sidx = scan*adj - 1 (invalid slots -> -1, ignored by local_scatter)
     compacted into dense per-row neighbor lists by GPSIMD local_scatter.
  3. One bf16 dma_gather per row-tile (4224 rows x 512B), alternating SWDGE
     queues so two gathers run concurrently.
  4. GATv2 attention with exp-softmax (|e| small, no max subtraction),
     bf16 elementwise math, f32 softmax/LayerNorm, wide fused-AP ops.
"""

import numpy as np

import concourse.bass as bass
import concourse.bacc as bacc
import concourse.mybir as mybir
from concourse import tile
from concourse.bass_utils import run_bass_kernel_spmd

F32 = mybir.dt.float32
BF16 = mybir.dt.bfloat16
I16 = mybir.dt.int16
Alu = mybir.AluOpType
Act = mybir.ActivationFunctionType
X = mybir.AxisListType.X

B, N, K, IN, H, D = 4, 4096, 32, 64, 4, 64
HD = H * D  # 256
NEG_SLOPE = 0.2
EPS = 1e-5

NCORES = 8
T = N // 2  # 2048 targets per core
NT = T // 128  # 16 target tiles
KS = K + 1  # 33 gather slots per target (<=32 used + pad)
SLOT = KS  # per-row slot stride in the packed neighbor list
JJ = 2  # row-tiles per block
NBLK = NT // JJ
W8 = KS * 8  # wrapped index columns per tile

_cache = {}
TIME_REPEAT = 9
BEST_MODE = "full"  # production variant used by kernel(); timing uses it too


def ap_of(t):
    return t if isinstance(t, bass.AP) else t[:]


def rap(t, pairs, extra_off=0):
    """AP on tile/AP `t`: keep partition dim, set free [step, count] pairs
    (element units), optionally add an element offset."""
    a = ap_of(t)
    return bass.AP(tensor=a.tensor, offset=a.offset + extra_off,
                   ap=[a.ap[0], *pairs])


def build_program(repeat=1, mode="full"):
    nc = bacc.Bacc("TRN2", target_bir_lowering=False, debug=False,
                   num_devices=NCORES,
                   num_swdge_queues=2 if mode == "dq" else 1)

    adj_d = nc.dram_tensor("adj", [T, N], F32, kind="ExternalInput")
    x_d = nc.dram_tensor("x", [N, IN], F32, kind="ExternalInput")
    xh_d = nc.dram_tensor("xh", [T, IN], F32, kind="ExternalInput")
    wl_d = nc.dram_tensor("Wl", [IN, HD], F32, kind="ExternalInput")
    bl_d = nc.dram_tensor("bl", [HD], F32, kind="ExternalInput")
    wr_d = nc.dram_tensor("Wr", [IN, HD], F32, kind="ExternalInput")
    br_d = nc.dram_tensor("br", [HD], F32, kind="ExternalInput")
    att_d = nc.dram_tensor("attv", [HD], F32, kind="ExternalInput")
    bias_d = nc.dram_tensor("bias", [HD], F32, kind="ExternalInput")
    gamma_d = nc.dram_tensor("gamma", [HD], F32, kind="ExternalInput")
    beta_d = nc.dram_tensor("beta", [HD], F32, kind="ExternalInput")
    y_d = nc.dram_tensor("y", [T, HD], F32, kind="ExternalOutput")
    xl_d = nc.dram_tensor("xl_scratch", [N, HD], BF16)
    wrap_d = nc.dram_tensor("wrap_scratch", [16 * NT * W8], I16)

    with tile.TileContext(nc) as tc:
        _emit(nc, tc, locals(), repeat, mode)
    nc.compile()
    return nc


def _emit(nc, tc, io, repeat, mode="full"):
    adj_d, x_d, xh_d, y_d, xl_d, wrap_d = (
        io[k] for k in ("adj_d", "x_d", "xh_d", "y_d", "xl_d", "wrap_d"))

    from contextlib import ExitStack
    ctx = ExitStack()
    with ctx:
        consts = ctx.enter_context(tc.tile_pool(name="consts", bufs=1))

        def bconst(dram_t, tag, dt=F32):
            t = consts.tile([128, HD], dt, tag=tag)
            if dt == F32:
                nc.sync.dma_start(
                    out=t[:], in_=bass.AP(tensor=dram_t, offset=0,
                                          ap=[[0, 128], [1, HD]]))
            else:
                stage = consts.tile([128, HD], F32, tag=tag + "_st")
                nc.sync.dma_start(
                    out=stage[:], in_=bass.AP(tensor=dram_t, offset=0,
                                              ap=[[0, 128], [1, HD]]))
                nc.vector.tensor_copy(out=t[:], in_=stage[:])
            return t

        att_bb = bconst(io["att_d"], "att_bb", BF16)
        bias_b = bconst(io["bias_d"], "bias_b")
        gamma_b = bconst(io["gamma_d"], "gamma_b")
        beta_b = bconst(io["beta_d"], "beta_b")

        eps_t = consts.tile([128, 1], F32)
        nc.vector.memset(eps_t[:], EPS)
        zerob = consts.tile([128, 1], BF16)
        nc.vector.memset(zerob[:], 0.0)

        iota_tmp = consts.tile([128, KS], I16)
        nc.gpsimd.iota(iota_tmp[:], pattern=[[1, KS]], base=0,
                       channel_multiplier=0)
        iota_kf = consts.tile([128, KS], F32)
        nc.vector.tensor_copy(out=iota_kf[:], in_=iota_tmp[:])

        iota_j = consts.tile([128, JJ * N], I16)
        nc.gpsimd.iota(iota_j[:], pattern=[[0, JJ], [1, N]], base=0,
                       channel_multiplier=0)

        gd = None
        if mode == "nodep":
            gd = consts.tile([128, KS, HD], BF16)
            nc.vector.memset(gd[:], 0.25)

        # persistent per-core state (per-rep tensors double-buffered so
        # extraction of rep r+1 can overlap attention of rep r)
        xr_all = consts.tile([128, NT, HD], BF16)
        xlh_all = consts.tile([128, NT, HD], BF16)
        wrap128_2 = [consts.tile([128, NT * W8], I16, name=f"wrap128_{i}")
                     for i in range(2)]
        nbr_all_2 = [consts.tile([128, NT * SLOT], I16, name=f"nbr_all_{i}")
                     for i in range(2)]
        cnt_all_2 = [consts.tile([128, NT], F32, name=f"cnt_all_{i}")
                     for i in range(2)]
        wself_all_2 = [consts.tile([128, NT, H], F32, name=f"wself_all_{i}")
                       for i in range(2)]

        # ---- setup: xl table in DRAM (bias via ones-row); xr, xlh on PE ----
        with tc.tile_pool(name="setup", bufs=2) as setup, \
             tc.tile_pool(name="setup_ps", bufs=4, space="PSUM") as setup_ps:
            xTa = setup.tile([IN + 1, N], F32)
            nc.sync.dma_start(
                out=xTa[:IN, :],
                in_=bass.AP(tensor=x_d, offset=0, ap=[[1, IN], [IN, N]]))
            nc.vector.memset(xTa[IN:IN + 1, :], 1.0)
            xhTa = setup.tile([IN + 1, T], F32)
            nc.sync.dma_start(
                out=xhTa[:IN, :],
                in_=bass.AP(tensor=xh_d, offset=0, ap=[[1, IN], [IN, T]]))
            nc.vector.memset(xhTa[IN:IN + 1, :], 1.0)

            wla = setup.tile([IN + 1, HD], F32)
            nc.sync.dma_start(out=wla[:IN, :], in_=io["wl_d"].ap())
            nc.sync.dma_start(out=wla[IN:IN + 1, :],
                              in_=bass.AP(tensor=io["bl_d"], offset=0,
                                          ap=[[0, 1], [1, HD]]))
            wra = setup.tile([IN + 1, HD], F32)
            nc.sync.dma_start(out=wra[:IN, :], in_=io["wr_d"].ap())
            nc.sync.dma_start(out=wra[IN:IN + 1, :],
                              in_=bass.AP(tensor=io["br_d"], offset=0,
                                          ap=[[0, 1], [1, HD]]))

            for grp in range(N // 512):  # xl -> DRAM, 4 node-chunks per DMA
                ps = setup_ps.tile([128, 4, HD], F32, tag="mm")
                for c4 in range(4):
                    c = grp * 4 + c4
                    nc.tensor.matmul(out=ps[:, c4, :],
                                     lhsT=xTa[:, c * 128:(c + 1) * 128],
                                     rhs=wla[:], start=True, stop=True)
                xls = setup.tile([128, 4, HD], BF16, tag="xls")
                nc.vector.tensor_copy(out=xls[:], in_=ps[:])
                nc.sync.dma_start(
                    out=bass.AP(tensor=xl_d, offset=grp * 512 * HD,
                                ap=[[HD, 128], [128 * HD, 4], [1, HD]]),
                    in_=xls[:])

            for grp in range(NT // 4):  # xr and xlh (self rows), SBUF bf16
                ps = setup_ps.tile([128, 4, HD], F32, tag="mm")
                for c4 in range(4):
                    c = grp * 4 + c4
                    nc.tensor.matmul(out=ps[:, c4, :],
                                     lhsT=xhTa[:, c * 128:(c + 1) * 128],
                                     rhs=wra[:], start=True, stop=True)
                nc.vector.tensor_copy(out=xr_all[:, grp * 4:(grp + 1) * 4, :],
                                      in_=ps[:])
                ps2 = setup_ps.tile([128, 4, HD], F32, tag="mm")
                for c4 in range(4):
                    c = grp * 4 + c4
                    nc.tensor.matmul(out=ps2[:, c4, :],
                                     lhsT=xhTa[:, c * 128:(c + 1) * 128],
                                     rhs=wla[:], start=True, stop=True)
                nc.vector.tensor_copy(out=xlh_all[:, grp * 4:(grp + 1) * 4, :],
                                      in_=ps2[:])

        # ---- main (repeatable for timing) ---------------------------------
        smallp = ctx.enter_context(tc.tile_pool(name="smallp", bufs=2))
        outp = ctx.enter_context(tc.tile_pool(name="outp", bufs=2))

        repp = tc.alloc_tile_pool(name="repp", bufs=1)
        p1 = tc.alloc_tile_pool(name="p1", bufs=1)
        adjp = tc.alloc_tile_pool(name="adjp", bufs=2)
        gpA = tc.alloc_tile_pool(name="gpA", bufs=2)
        gpB = tc.alloc_tile_pool(name="gpB", bufs=2)
        auxp = tc.alloc_tile_pool(name="auxp", bufs=1)
        for _rep in range(repeat):
            wrap128 = wrap128_2[_rep % 2]
            nbr_all = nbr_all_2[_rep % 2]
            cnt_all = cnt_all_2[_rep % 2]
            wself_all = wself_all_2[_rep % 2]
            # ---- phase 1: neighbor extraction per 2-tile block -------------
            for blk in range(NBLK):
                t0 = blk * JJ
                adj_t = [adjp.tile([128, N], BF16, tag="adj",
                                   name=f"adj{blk}_{j}") for j in (0, 1)]
                if mode in ("synadj", "dq"):
                    for j in range(JJ):
                        adj_f32 = adjp.tile([128, N], F32, tag="adjf",
                                            name=f"adjf{blk}_{j}")
                        nc.sync.dma_start(
                            out=adj_f32[:],
                            in_=bass.AP(tensor=adj_d,
                                        offset=(t0 + j) * 128 * N,
                                        ap=[[N, 128], [1, N]]))
                        nc.vector.tensor_copy(out=adj_t[j][:],
                                              in_=adj_f32[:])
                else:
                    for j in range(JJ):
                        nc.gpsimd.dma_start(
                            out=adj_t[j][:],
                            in_=bass.AP(tensor=adj_d,
                                        offset=(t0 + j) * 128 * N,
                                        ap=[[N, 128], [1, N]]))

                s = p1.tile([128, JJ, N], BF16, tag="scan")
                for j in range(JJ):
                    nc.vector.tensor_tensor_scan(
                        out=s[:, j, :], data0=adj_t[j][:],
                        data1=rap(zerob, [[0, N]]),
                        initial=float(j * SLOT), op0=Alu.add, op1=Alu.add)
                s_f = s[:].rearrange("p j n -> p (j n)")

                rowlast = smallp.tile([128, JJ], F32, tag="rowlast")
                nc.vector.tensor_copy(out=rowlast[:],
                                      in_=rap(s_f, [[N, JJ]],
                                              extra_off=N - 1))
                nc.vector.tensor_copy(out=cnt_all[:, t0:t0 + 1],
                                      in_=rowlast[:, 0:1])
                nc.vector.tensor_scalar_add(out=cnt_all[:, t0 + 1:t0 + 2],
                                            in0=rowlast[:, 1:2],
                                            scalar1=-float(SLOT))
                # sidx = s*adj - 1: valid -> rank-1 + j*SLOT, invalid -> -1
                for j in range(JJ):
                    nc.vector.tensor_tensor(out=s[:, j, :], in0=s[:, j, :],
                                            in1=adj_t[j][:], op=Alu.mult)
                sidx = s[:].bitcast(I16).rearrange("p j n -> p (j n)")
                nc.vector.tensor_scalar_add(out=sidx, in0=s_f, scalar1=-1.0)
                nc.gpsimd.local_scatter(
                    out_ap=nbr_all[:, t0 * SLOT:(t0 + JJ) * SLOT],
                    data_ap=iota_j[:], idxs_ap=sidx, channels=128,
                    num_elems=JJ * SLOT, num_idxs=JJ * N)

            if mode == "p1only":
                oc = outp.tile([128, NT], F32, tag="oc")
                nc.vector.tensor_copy(out=oc[:], in_=cnt_all[:])
                nc.sync.dma_start(
                    out=bass.AP(tensor=y_d, offset=0,
                                ap=[[HD, 128], [1, NT]]),
                    in_=oc[:])
                continue

            # ---- wrapped index layout for dma_gather -----------------------
            # edge e = k*128+p of tile t -> wrapped[p%16, t*W8 + k*8 + p//16]
            wrapT = repp.tile([16, NT, KS, 8], I16, tag="wrapT")
            for ph in range(8):
                nc.sync.dma_start(
                    out=rap(wrapT, [[8, NT * KS]], extra_off=ph),
                    in_=nbr_all[ph * 16:(ph + 1) * 16, :])
            nc.sync.dma_start(out=wrap_d.ap(),
                              in_=wrapT[:].rearrange("p t k e -> p (t k e)"))
            nc.sync.dma_start(
                out=wrap128[:],
                in_=bass.AP(tensor=wrap_d, offset=0,
                            ap=[[0, 8], [NT * W8, 16], [1, NT * W8]]))

            # ---- self scores: w_self = exp(<att, lrelu(xlh + xr)>) ---------
            if mode == "novec":
                nc.vector.memset(wself_all[:], 1.0)
            else:
                zs = repp.tile([128, NT, HD], BF16, tag="zs")
                nc.vector.tensor_tensor(out=zs[:], in0=xlh_all[:],
                                        in1=xr_all[:], op=Alu.add)
                nc.vector.scalar_tensor_tensor(out=zs[:], in0=zs[:],
                                               scalar=NEG_SLOPE, op0=Alu.mult,
                                               in1=zs[:], op1=Alu.max)
                nc.vector.tensor_tensor(out=zs[:], in0=zs[:],
                                        in1=rap(att_bb, [[0, NT], [1, HD]]),
                                        op=Alu.mult)
                es = smallp.tile([128, NT * H], F32, tag="eself")
                nc.vector.tensor_reduce(
                    out=es[:], in_=rap(zs, [[D, NT * H], [1, D]]),
                    op=Alu.add, axis=X)
                nc.scalar.activation(out=wself_all[:], in_=es[:],
                                     func=Act.Exp)

            # ---- phase 2: gather + attention per 2-tile block --------------
            # software pipeline: issue block b+1's gathers before block b's
            # vector chain so the Pool/DMA stream runs a block ahead of DVE.
            def emit_gather(blk):
                t0 = blk * JJ
                gpp = gpA if blk % 2 == 0 else gpB
                gt = [gpp.tile([128, KS, HD], BF16, tag="g",
                               name=f"g{blk}_{jj}") for jj in range(JJ)]
                if mode == "nogather":
                    if _rep == 0 and blk < 2:
                        for jj in range(JJ):
                            nc.vector.memset(gt[jj][:], 0.25)
                else:
                    for jj in range(JJ):
                        t = t0 + jj
                        nc.gpsimd.dma_gather(
                            out_ap=gt[jj][:], in_ap=xl_d.ap(),
                            idxs_ap=wrap128[:, t * W8:(t + 1) * W8],
                            num_idxs=KS * 128, num_idxs_reg=KS * 128,
                            elem_size=HD, single_packet=False,
                            queue_num=jj if mode == "dq" else 0)
                return gt

            pending = emit_gather(0)
            for blk in range(NBLK):
                t0 = blk * JJ
                gt = pending
                if blk + 1 < NBLK:
                    pending = emit_gather(blk + 1)
                gj = [gt[jj][:] for jj in range(JJ)]

                if mode == "novec":
                    o = outp.tile([128, JJ, HD], F32, tag="o")
                    for jj in range(JJ):
                        nc.vector.tensor_copy(out=o[:, jj, :],
                                              in_=gt[jj][:, 0, :])
                    nc.sync.dma_start(
                        out=bass.AP(tensor=y_d, offset=t0 * 128 * HD,
                                    ap=[[HD, 128], [128 * HD, JJ], [1, HD]]),
                        in_=o[:])
                    continue

                if mode == "nodep":
                    gj = [gd[:], gd[:]]
                e = smallp.tile([128, JJ, KS * H], F32, tag="e")
                for jj in range(JJ):
                    aux = auxp.tile([128, KS, HD], BF16, tag="aux")
                    nc.vector.tensor_tensor(
                        out=aux[:], in0=gj[jj],
                        in1=rap(xr_all, [[0, KS], [1, HD]],
                                extra_off=(t0 + jj) * HD),
                        op=Alu.add)
                    nc.vector.scalar_tensor_tensor(
                        out=aux[:], in0=aux[:], scalar=NEG_SLOPE,
                        op0=Alu.mult, in1=aux[:], op1=Alu.max)
                    nc.vector.tensor_tensor(
                        out=aux[:], in0=aux[:],
                        in1=rap(att_bb, [[0, KS], [1, HD]]),
                        op=Alu.mult)
                    nc.vector.tensor_reduce(
                        out=e[:, jj, :],
                        in_=rap(aux, [[D, KS * H], [1, D]]),
                        op=Alu.add, axis=X)
                w = smallp.tile([128, JJ, KS, H], F32, tag="w")
                nc.scalar.activation(out=w[:], in_=e[:], func=Act.Exp)
                kmask = smallp.tile([128, JJ, KS], F32, tag="kmask")
                nc.vector.tensor_tensor(
                    out=kmask[:], in0=rap(iota_kf, [[0, JJ], [1, KS]]),
                    in1=rap(cnt_all, [[1, JJ], [0, KS]], extra_off=t0),
                    op=Alu.is_lt)
                nc.vector.tensor_tensor(
                    out=w[:], in0=w[:],
                    in1=rap(kmask, [[KS, JJ], [1, KS], [0, H]]),
                    op=Alu.mult)
                zsum = smallp.tile([128, JJ, H], F32, tag="zsum")
                nc.vector.tensor_reduce(
                    out=zsum[:], in_=rap(w, [[KS * H, JJ], [1, H], [H, KS]]),
                    op=Alu.add, axis=X)
                nc.vector.tensor_tensor(out=zsum[:], in0=zsum[:],
                                        in1=wself_all[:, t0:t0 + JJ, :],
                                        op=Alu.add)
                rz = smallp.tile([128, JJ, H], F32, tag="rz")
                nc.vector.reciprocal(out=rz[:], in_=zsum[:])
                w2b = smallp.tile([128, JJ, KS, H], BF16, tag="w2b")
                nc.vector.tensor_tensor(
                    out=w2b[:], in0=w[:],
                    in1=rap(rz, [[H, JJ], [0, KS], [1, H]]),
                    op=Alu.mult)
                o = outp.tile([128, JJ, HD], F32, tag="o")
                for jj in range(JJ):
                    nc.vector.tensor_tensor(
                        out=gj[jj], in0=gj[jj],
                        in1=rap(w2b[:, jj, :, :], [[H, KS], [1, H], [0, D]]),
                        op=Alu.mult)
                    nc.vector.tensor_reduce(
                        out=o[:, jj, :],
                        in_=rap(gj[jj],
                                [[D, H], [1, D], [H * D, KS]]),
                        op=Alu.add, axis=X)
                # self contribution: o += xlh * (wself * rz)
                wsn = smallp.tile([128, JJ, H], BF16, tag="wsn")
                nc.vector.tensor_tensor(out=wsn[:],
                                        in0=wself_all[:, t0:t0 + JJ, :],
                                        in1=rz[:], op=Alu.mult)
                sc = smallp.tile([128, JJ, HD], F32, tag="sc")
                nc.vector.tensor_tensor(
                    out=sc[:], in0=xlh_all[:, t0:t0 + JJ, :],
                    in1=rap(wsn, [[H, JJ], [1, H], [0, D]]), op=Alu.mult)
                nc.vector.tensor_tensor(out=o[:], in0=o[:], in1=sc[:],
                                        op=Alu.add)
                nc.vector.tensor_tensor(
                    out=o[:], in0=o[:],
                    in1=rap(bias_b, [[0, JJ], [1, HD]]), op=Alu.add)

                # LayerNorm over HD
                stats = smallp.tile([128, JJ, 6], F32, tag="stats")
                mv = smallp.tile([128, JJ, 2], F32, tag="mv")
                for jj in range(JJ):
                    nc.vector.bn_stats(out=stats[:, jj, :], in_=o[:, jj, :])
                    nc.vector.bn_aggr(out=mv[:, jj, :], in_=stats[:, jj, :])
                ve = smallp.tile([128, JJ], F32, tag="ve")
                nc.vector.tensor_tensor(out=ve[:],
                                        in0=rap(mv, [[2, JJ]], extra_off=1),
                                        in1=rap(eps_t, [[0, JJ]]), op=Alu.add)
                lnv = smallp.tile([128, JJ], F32, tag="lnv")
                nc.scalar.activation(out=lnv[:], in_=ve[:], func=Act.Ln)
                rstd = smallp.tile([128, JJ], F32, tag="rstd")
                nc.scalar.activation(out=rstd[:], in_=lnv[:], func=Act.Exp,
                                     scale=-0.5)
                for jj in range(JJ):
                    nc.vector.scalar_tensor_tensor(
                        out=o[:, jj, :], in0=o[:, jj, :],
                        scalar=mv[:, jj, 0:1], op0=Alu.subtract,
                        in1=rap(rstd, [[0, HD]], extra_off=jj), op1=Alu.mult)
                nc.vector.tensor_tensor(
                    out=o[:], in0=o[:],
                    in1=rap(gamma_b, [[0, JJ], [1, HD]]), op=Alu.mult)
                nc.vector.tensor_tensor(
                    out=o[:], in0=o[:],
                    in1=rap(beta_b, [[0, JJ], [1, HD]]), op=Alu.add)
                nc.sync.dma_start(
                    out=bass.AP(tensor=y_d, offset=t0 * 128 * HD,
                                ap=[[HD, 128], [128 * HD, JJ], [1, HD]]),
                    in_=o[:])
        auxp.release()
        gpB.release()
        gpA.release()
        adjp.release()
        p1.release()
        repp.release()


def make_in_maps(inputs):
    adj = np.ascontiguousarray(inputs["adj"], np.float32)
    x = np.ascontiguousarray(inputs["x"], np.float32)
    flat = {
        "Wl": np.ascontiguousarray(inputs["Wl"], np.float32),
        "bl": np.ascontiguousarray(inputs["bl"], np.float32),
        "Wr": np.ascontiguousarray(inputs["Wr"], np.float32),
        "br": np.ascontiguousarray(inputs["br"], np.float32),
        "attv": np.ascontiguousarray(inputs["att"], np.float32).reshape(HD),
        "bias": np.ascontiguousarray(inputs["bias"], np.float32),
        "gamma": np.ascontiguousarray(inputs["gamma"], np.float32),
        "beta": np.ascontiguousarray(inputs["beta"], np.float32),
    }
    in_maps = []
    for c in range(NCORES):
        b, h = c // 2, c % 2
        in_maps.append({
            "adj": np.ascontiguousarray(adj[b, h * T:(h + 1) * T, :]),
            "x": np.ascontiguousarray(x[b]),
            "xh": np.ascontiguousarray(x[b, h * T:(h + 1) * T, :]),
            **flat,
        })
    return in_maps


def kernel(**inputs) -> np.ndarray:
    if "nc" not in _cache:
        _cache["nc"] = build_program(mode=BEST_MODE)
    nc = _cache["nc"]
    res = run_bass_kernel_spmd(nc, make_in_maps(inputs), list(range(NCORES)))
    y = np.zeros((B, N, HD), np.float32)
    for c in range(NCORES):
        b, h = c // 2, c % 2
        y[b, h * T:(h + 1) * T, :] = res.results[c]["y"]
    return y
